# revision 1
# baseline (speedup 1.0000x reference)
"""Trainium2 Bass kernel for nn_Attention_46995532153449.

Module: qkv = x @ w_qkv; per-head scores = q k^T * hd^-0.5; softmax over the
HEAD axis (axis=1); attn = probs @ v; out = attn @ w_proj + b_proj.

Shapes: B=2, T=2048, D=1024, H=16, HD=64.

Sharding: data-parallel over (batch, query-block). Core c handles batch
c // 4 and queries [(c % 4) * 512, (c % 4 + 1) * 512). The head-axis softmax
is local because every core holds all 16 heads for its query slice. Each
core recomputes K/V for its whole batch (replicated across the 4 cores of a
batch) so no collectives are needed.

Layout choices (all picked so that no on-chip transpose is ever required,
and so that every matmul is a full-128-partition matmul — operands at
base_partition 64 fail on this hardware):
  - host feeds x^T (fp16), so QKV projections produce q^T/k^T [e, t] with
    e on partitions (lhsT = W as-is, rhs = x^T) and v [t, e] (lhsT = x^T
    tile, rhs = Wv).
  - scores^T[k, q] per head via a zero-padded q^T (qpad): for head pair pr,
    columns [0:QH] hold head 2pr's q^T at partitions 0:64 (zeros at
    64:128) and columns [QH:2QH] hold head 2pr+1's at partitions 64:128.
    One K=128 matmul per pair (lhsT = k^T pair chunk, rhs = qpad) yields
    both heads' scores^T side by side. ScalarE evacuates the scores PSUM
    with a fused scale+exp into fp16 E tiles.
  - head-axis softmax: S = sum of the 16 E tiles (VectorE log-tree),
    R = 1/S (VectorE reciprocal), P = E * R broadcast — split across
    VectorE (heads 0:8) and GpSimd (heads 8:16) to balance engine load.
  - attn^T[d, q] = v^T P^T per head: lhsT = v tile [k, 64], rhs = P^T
    [k, q]; odd heads write output partitions 64:128 (col-tiled matmuls,
    concurrent with the even head's). Per-head PSUM accumulation groups
    share a bank partition-split (verified on HW: has_written clearing is
    per partition; the simulator's bank-granular group check is skipped
    via skip_group_check). Accumulated over KB=4 key-chunk blocks in
    PSUM, then spill-added into an SBUF fp32 accumulator on VectorE.
  - out[q, e]: lhsT = attn^T tile [d, q], rhs = w_proj [d, e]. Output is in
    natural [q, e] order for a contiguous DMA; bias added during PSUM
    evacuation.

Measured on the 8-core axon trn2 target: max rel err 6.7e-4 vs a float64
reference; cost-model timeline estimate ~394 us/core.
"""

import numpy as np

import concourse.bacc as bacc
import concourse.mybir as mybir
import concourse.tile as tile
from concourse import bass_utils

B, T, D, H = 2, 2048, 1024, 16
HD = D // H          # 64
SCALE = HD ** -0.5   # 0.125
NCORES = 8
QS = B * T // NCORES  # 512 queries per core
DC = D // 128         # 8 d/e chunks of 128
TC = T // 128         # 16 key chunks of 128
QH = QS // 2          # 256, query half (PSUM budget)
KB = 4                # key chunks per attention block
NBLK = TC // KB

F16 = mybir.dt.float16
F32 = mybir.dt.float32
ADD = mybir.AluOpType.add
MULT = mybir.AluOpType.mult
EXP = mybir.ActivationFunctionType.Exp

_CACHED_NC = None


def _build_nc():
    nc = bacc.Bacc(
        "TRN2", target_bir_lowering=False, debug=False, enable_asserts=False
    )

    xT_d = nc.dram_tensor("xt", [D, T], F16, kind="ExternalInput").ap()
    xTq_d = nc.dram_tensor("xtq", [D, QS], F16, kind="ExternalInput").ap()
    wq_d = nc.dram_tensor("wq", [D, D], F16, kind="ExternalInput").ap()
    wk_d = nc.dram_tensor("wk", [D, D], F16, kind="ExternalInput").ap()
    wv_d = nc.dram_tensor("wv", [D, D], F16, kind="ExternalInput").ap()
    wp_d = nc.dram_tensor("wp", [D, D], F16, kind="ExternalInput").ap()
    bias_d = nc.dram_tensor("bias", [128, D], F32, kind="ExternalInput").ap()
    out_d = nc.dram_tensor("out", [QS, D], F32, kind="ExternalOutput").ap()

    def chunked(ap):  # [(c p), f] -> [p, c, f]
        return ap.rearrange("(c p) f -> p c f", p=128)

    with tile.TileContext(nc) as tc:
        with tc.tile_pool(name="persist", bufs=1) as pp:
            kT = pp.tile([128, DC, T], F16)      # k^T: [e, t], e-chunk major
            v_sb = pp.tile([128, TC, D], F16)    # v: [t, e], t-chunk major
            # zero-padded q^T: for head pair pr and query half sel, columns
            # [0:QH] hold head 2pr's q^T at partitions 0:64 (zeros below),
            # columns [QH:2QH] hold head 2pr+1's at partitions 64:128.
            # Keeps every scores matmul a full-128-partition K=128 matmul
            # (operands at base_partition 64 fail on hardware).
            qpad = pp.tile([128, DC, 2, 2 * QH], F16)
            aT = pp.tile([128, DC, QS], F16)     # attn^T: [d, q]
            wp_sb = pp.tile([128, DC, D], F16)
            bi_sb = pp.tile([128, D], F32)

            nc.gpsimd.memset(qpad, 0.0)
            nc.sync.dma_start(wp_sb, chunked(wp_d))
            nc.sync.dma_start(bi_sb, bias_d)

            # ---------------- Phase 1: QKV projections ----------------
            with tc.tile_pool(name="ph1x", bufs=1) as p1x:
                xT = p1x.tile([128, DC, T], F16)

                with (
                    tc.tile_pool(name="ph1q", bufs=1) as p1q,
                    tc.tile_pool(name="ppsq", bufs=4, space="PSUM") as ppsq,
                ):
                    xTq = p1q.tile([128, DC, QS], F16)
                    wq_sb = p1q.tile([128, DC, D], F16)
                    # Q's inputs first: the DMA ring is FIFO and these gate
                    # the kernel's first matmuls; the big x^T transfer follows
                    nc.sync.dma_start(xTq, chunked(xTq_d))
                    nc.sync.dma_start(wq_sb, chunked(wq_d))
                    nc.sync.dma_start(xT, chunked(xT_d))
                    # q^T[e, q] for this core's q-slice, written into the
                    # zero-padded layout (4 partition/half-sliced copies)
                    for ej in range(DC):
                        ps = ppsq.tile([128, 512], F32, tag="ps")
                        for jd in range(DC):
                            nc.tensor.matmul(
                                ps,
                                lhsT=wq_sb[:, jd, ej * 128:(ej + 1) * 128],
                                rhs=xTq[:, jd, :],
                                start=(jd == 0),
                                stop=(jd == DC - 1),
                            )
                        for sel in range(2):
                            nc.scalar.copy(
                                qpad[0:64, ej, sel, 0:QH],
                                ps[0:64, sel * QH:(sel + 1) * QH],
                            )
                            nc.scalar.copy(
                                qpad[64:128, ej, sel, QH:2 * QH],
                                ps[64:128, sel * QH:(sel + 1) * QH],
                            )

                with (
                    tc.tile_pool(name="ph1k", bufs=1) as p1k,
                    tc.tile_pool(name="ppsk", bufs=4, space="PSUM") as ppsk,
                ):
                    wk_sb = p1k.tile([128, DC, D], F16)
                    nc.sync.dma_start(wk_sb, chunked(wk_d))
                    # k^T[e, t] for the whole batch (tj outer: early key
                    # chunks complete first so attention can start sooner)
                    for tj in range(T // 512):
                        for ej in range(DC):
                            ps = ppsk.tile([128, 512], F32, tag="ps")
                            for jd in range(DC):
                                nc.tensor.matmul(
                                    ps,
                                    lhsT=wk_sb[:, jd, ej * 128:(ej + 1) * 128],
                                    rhs=xT[:, jd, tj * 512:(tj + 1) * 512],
                                    start=(jd == 0),
                                    stop=(jd == DC - 1),
                                )
                            nc.scalar.copy(
                                kT[:, ej, tj * 512:(tj + 1) * 512], ps
                            )

                with (
                    tc.tile_pool(name="ph1v", bufs=1) as p1v,
                    tc.tile_pool(name="ppsv", bufs=4, space="PSUM") as ppsv,
                ):
                    wv_sb = p1v.tile([128, DC, D], F16)
                    nc.sync.dma_start(wv_sb, chunked(wv_d))
                    # v[t, e] for the whole batch
                    for tj in range(TC):
                        for eh in range(2):
                            ps = ppsv.tile([128, 512], F32, tag="ps")
                            for jd in range(DC):
                                nc.tensor.matmul(
                                    ps,
                                    lhsT=xT[:, jd, tj * 128:(tj + 1) * 128],
                                    rhs=wv_sb[:, jd, eh * 512:(eh + 1) * 512],
                                    start=(jd == 0),
                                    stop=(jd == DC - 1),
                                )
                            nc.vector.tensor_copy(
                                v_sb[:, tj, eh * 512:(eh + 1) * 512], ps
                            )

            # ---------------- Phase 2: attention ----------------
            with (
                tc.tile_pool(name="attps", bufs=2, space="PSUM") as aps,
                tc.tile_pool(name="scps", bufs=2, space="PSUM") as sps,
                tc.tile_pool(name="ework", bufs=2) as epool,
                tc.tile_pool(name="swork", bufs=2) as spool,
                tc.tile_pool(name="accp", bufs=1) as accpool,
            ):
                for qh in range(2):
                    acc = accpool.tile([128, DC, QH], F32, tag="acc")
                    for blk in range(NBLK):
                        Eb = epool.tile([128, KB, H, QH], F16, tag="Eb")
                        for kcl in range(KB):
                            kc = blk * KB + kcl
                            for g in range(4):  # 4 heads per PSUM tile
                                sc = sps.tile([128, 4 * QH], F32, tag="sc")
                                for i in range(2):  # head pairs 2g, 2g+1
                                    pr = 2 * g + i
                                    nc.tensor.matmul(
                                        sc[:, i * 2 * QH:(i + 1) * 2 * QH],
                                        lhsT=kT[:, pr,
                                                kc * 128:(kc + 1) * 128],
                                        rhs=qpad[:, pr, qh, :],
                                        start=True,
                                        stop=True,
                                    )
                                # fused PSUM evacuation + scale + exp
                                nc.scalar.activation(
                                    Eb[:, kcl, 4 * g:4 * g + 4, :],
                                    sc,
                                    EXP,
                                    scale=SCALE,
                                )
                            # S = sum over heads (log tree), R = 1/S, P = E*R
                            E = Eb[:, kcl]
                            tmp = spool.tile([128, H // 2, QH], F16, tag="tmp")
                            nc.vector.tensor_tensor(
                                tmp, E[:, 0:8], E[:, 8:16], ADD
                            )
                            nc.vector.tensor_tensor(
                                tmp[:, 0:4], tmp[:, 0:4], tmp[:, 4:8], ADD
                            )
                            nc.vector.tensor_tensor(
                                tmp[:, 0:2], tmp[:, 0:2], tmp[:, 2:4], ADD
                            )
                            nc.vector.tensor_tensor(
                                tmp[:, 0:1], tmp[:, 0:1], tmp[:, 1:2], ADD
                            )
                            r = spool.tile([128, 1, QH], F16, tag="r")
                            with nc.allow_low_precision(
                                reason="softmax denominator reciprocal in fp16"
                            ):
                                nc.vector.reciprocal(r, tmp[:, 0:1])
                            nc.vector.tensor_tensor(
                                E[:, 0:8], E[:, 0:8],
                                r.to_broadcast([128, 8, QH]), MULT
                            )
                            nc.gpsimd.tensor_tensor(
                                E[:, 8:16], E[:, 8:16],
                                r.to_broadcast([128, 8, QH]), MULT
                            )
                        # attn^T: 4 waves x 2 d-chunks; one accumulation
                        # group per full PSUM bank (128 partitions), two
                        # zero-padded per-head matmuls per key chunk. 2-bank
                        # wave tiles with bufs=2 so the next wave's matmuls
                        # overlap this wave's VectorE spill-add.
                        for w in range(4):
                            ps = aps.tile([128, 2, 2 * QH], F32, tag="wv")
                            for kcl in range(KB):
                                kc = blk * KB + kcl
                                for jdl in range(2):
                                    for par in range(2):
                                        h = 4 * w + 2 * jdl + par
                                        lo = par * 64
                                        nc.tensor.matmul(
                                            ps[lo:lo + 64, jdl, 0:QH],
                                            lhsT=v_sb[:, kc,
                                                      h * 64:(h + 1) * 64],
                                            rhs=Eb[:, kcl, h, :],
                                            start=(kcl == 0),
                                            stop=(kcl == KB - 1),
                                            skip_group_check=True,
                                        )
                            if blk == 0:
                                nc.vector.tensor_copy(
                                    acc[:, 2 * w:2 * w + 2, :], ps[:, :, 0:QH]
                                )
                            elif blk == NBLK - 1:
                                # final spill writes the fp16 attn^T tile
                                # directly (saves a ScalarE conversion pass)
                                nc.vector.tensor_tensor(
                                    aT[:, 2 * w:2 * w + 2,
                                       qh * QH:(qh + 1) * QH],
                                    ps[:, :, 0:QH],
                                    acc[:, 2 * w:2 * w + 2, :],
                                    ADD,
                                )
                            else:
                                nc.vector.tensor_tensor(
                                    acc[:, 2 * w:2 * w + 2, :],
                                    ps[:, :, 0:QH],
                                    acc[:, 2 * w:2 * w + 2, :],
                                    ADD,
                                )


            # ---------------- Phase 3: output projection ----------------
            out_ch = chunked(out_d)  # [128, QS//128, D]
            with (
                tc.tile_pool(name="prj", bufs=2, space="PSUM") as prj,
                tc.tile_pool(name="outp", bufs=2) as opool,
            ):
                for qs in range(QS // 128):
                    for eh in range(2):
                        pm = prj.tile([128, 512], F32, tag="pm")
                        for jd in range(DC):
                            nc.tensor.matmul(
                                pm,
                                lhsT=aT[:, jd, qs * 128:(qs + 1) * 128],
                                rhs=wp_sb[:, jd, eh * 512:(eh + 1) * 512],
                                start=(jd == 0),
                                stop=(jd == DC - 1),
                            )
                        ot = opool.tile([128, 512], F32, tag="ot")
                        nc.vector.tensor_tensor(
                            ot, pm, bi_sb[:, eh * 512:(eh + 1) * 512], ADD
                        )
                        nc.sync.dma_start(
                            out_ch[:, qs, eh * 512:(eh + 1) * 512], ot
                        )

    nc.compile()
    return nc


def get_nc():
    global _CACHED_NC
    if _CACHED_NC is None:
        _CACHED_NC = _build_nc()
    return _CACHED_NC


def kernel(x, w_qkv, w_proj, b_proj, _trace=False, _tmpdir=None):
    x = np.asarray(x, dtype=np.float32)
    w_qkv = np.asarray(w_qkv, dtype=np.float32)
    w_proj = np.asarray(w_proj, dtype=np.float32)
    b_proj = np.asarray(b_proj, dtype=np.float32)

    # Host-side layout prep: transpose + fp16 casts + shard.
    xT = [np.ascontiguousarray(x[b].T).astype(np.float16) for b in range(B)]
    wq = np.ascontiguousarray(w_qkv[:, 0:D]).astype(np.float16)
    wk = np.ascontiguousarray(w_qkv[:, D:2 * D]).astype(np.float16)
    wv = np.ascontiguousarray(w_qkv[:, 2 * D:3 * D]).astype(np.float16)
    wp = w_proj.astype(np.float16)
    bias = np.ascontiguousarray(
        np.broadcast_to(b_proj, (128, D))
    ).astype(np.float32)

    in_maps = []
    for c in range(NCORES):
        b = c // (NCORES // B)
        qofs = (c % (NCORES // B)) * QS
        in_maps.append(
            {
                "xt": xT[b],
                "xtq": np.ascontiguousarray(xT[b][:, qofs:qofs + QS]),
                "wq": wq,
                "wk": wk,
                "wv": wv,
                "wp": wp,
                "bias": bias,
            }
        )

    nc = get_nc()
    res = bass_utils.run_bass_kernel_spmd(
        nc,
        in_maps,
        core_ids=list(range(NCORES)),
        trace=_trace,
        tmpdir=_tmpdir,
    )

    out = np.empty((B, T, D), dtype=np.float32)
    for c in range(NCORES):
        b = c // (NCORES // B)
        qofs = (c % (NCORES // B)) * QS
        out[b, qofs:qofs + QS] = res.results[c]["out"]
    if _trace:
        kernel._last_results = res
    return out



# revision 6
# speedup vs baseline: 1.0031x; 1.0031x over previous
"""Trainium2 Bass kernel for nn_Attention_46995532153449.

Module: qkv = x @ w_qkv; per-head scores = q k^T * hd^-0.5; softmax over the
HEAD axis (axis=1); attn = probs @ v; out = attn @ w_proj + b_proj.

Shapes: B=2, T=2048, D=1024, H=16, HD=64.

Sharding: data-parallel over (batch, query-block): core c handles batch c//4
and queries [(c%4)*512, (c%4+1)*512). The head-axis softmax is local (each
core holds all 16 heads for its query slice). K/V for the whole batch are
recomputed per core (collectives are priced far above their compute saving
by the cost model, so no cross-core exchange).

Key layout/scheduling choices vs the earlier version of this kernel:
  - host feeds x^T fp16 with columns ROTATED so the core's own 512 queries
    are columns 0:512 (one SPMD program, per-core data). Key order is a
    rotation, which attention is invariant to.
  - K/V production is software-pipelined INTO the first attention pass
    (key-chunk kc consumes K/V right after they are produced), keeping
    TensorE saturated instead of a serial projection phase.
  - PV uses the attn[q, d] orientation: lhsT = P^T tile [keys, q] (M=128),
    rhs = v [keys, 64] (N=64) -> full 128-partition output, half the PE
    cycles of the attn^T[d, q] orientation. Accumulation runs over all 16
    key chunks directly in PSUM (4 banks per 256-query half), eliminating
    all VectorE spill-adds.
  - attention is two passes over key chunks (qh = 256-query halves) to fit
    PSUM: pass A also produces K/V; pass B overlaps the output projection
    of the first half.
  - attn[q, d] -> attn^T[d, q] via dma_start_transpose (xbar): costs DMA
    cycles only, no PE/ACT/DVE time, no PSUM.
  - softmax: exp on ScalarE (fused scale, PSUM evacuation); head-sum tree +
    reciprocal on VectorE; P = E*R split VectorE/GpSimd in pass B (all on
    VectorE in pass A where it has slack).

Measured via TimelineSim (the harness timing): see test.py output.
"""

import numpy as np

import concourse.bacc as bacc
import concourse.mybir as mybir
import concourse.tile as tile
from concourse import bass_utils

B, T, D, H = 2, 2048, 1024, 16
HD = D // H          # 64
SCALE = HD ** -0.5   # 0.125
NCORES = 8
QS = B * T // NCORES  # 512 queries per core
QH = QS // 2          # 256-query halves (PSUM budget)
DC = D // 128         # 8 d/e chunks of 128
TC = T // 128         # 16 key chunks of 128

F16 = mybir.dt.float16
F32 = mybir.dt.float32
ADD = mybir.AluOpType.add
MULT = mybir.AluOpType.mult
EXP = mybir.ActivationFunctionType.Exp

_CACHED_NC = None


def _build_nc():
    nc = bacc.Bacc(
        "TRN2", target_bir_lowering=False, debug=False, enable_asserts=False
    )

    xt_d = nc.dram_tensor("xt", [D, T], F16, kind="ExternalInput").ap()
    wq_d = nc.dram_tensor("wq", [D, D], F16, kind="ExternalInput").ap()
    wk_d = nc.dram_tensor("wk", [D, D], F16, kind="ExternalInput").ap()
    wv_d = nc.dram_tensor("wv", [D, D], F16, kind="ExternalInput").ap()
    wp_d = nc.dram_tensor("wp", [D, D], F16, kind="ExternalInput").ap()
    bias_d = nc.dram_tensor("bias", [128, D], F32, kind="ExternalInput").ap()
    out_d = nc.dram_tensor("out", [QS, D], F32, kind="ExternalOutput").ap()

    def chunked(ap):  # [(c p), f] -> [p, c, f]
        return ap.rearrange("(c p) f -> p c f", p=128)

    xt_ch = chunked(xt_d)
    out_ch = chunked(out_d)

    with tile.TileContext(nc) as tc:
        with tc.tile_pool(name="persist", bufs=1) as pp:
            kT = pp.tile([128, DC, T], F16)      # k^T: [e, t], e-chunk major
            v_sb = pp.tile([128, TC, D], F16)    # v: [t, e], t-chunk major
            # zero-padded q^T: for head pair pr and query half qh, columns
            # [0:QH] hold head 2pr's q^T at partitions 0:64 (zeros below),
            # columns [QH:2QH] hold head 2pr+1's at partitions 64:128, so
            # every scores matmul is a full-128-partition K=128 matmul.
            qpad = pp.tile([128, DC, 2, 2 * QH], F16)
            att = pp.tile([128, 2, D], F16)      # attn [q, d], per-qh reuse
            aT = pp.tile([128, DC, QS], F16)     # attn^T [d, q]
            wp_sb = pp.tile([128, DC, D], F16)
            bi_sb = pp.tile([128, D], F32)

            nc.gpsimd.memset(qpad, 0.0)

            with tc.tile_pool(name="pA", bufs=1) as pA:
                xT = pA.tile([128, DC, T], F16)
                wk_sb = pA.tile([128, DC, D], F16)
                wv_sb = pA.tile([128, DC, D], F16)

                # DMA ring order: own x^T piece -> wq -> wk -> rest of x^T
                # -> wv -> wp -> bias. Q gates on the first two; K production
                # on wk + successive x^T pieces; wp/bias only matter at the
                # output projection.
                nc.sync.dma_start(xT[:, :, 0:512], xt_ch[:, :, 0:512])

                with (
                    tc.tile_pool(name="qpool", bufs=1) as qp,
                    tc.tile_pool(name="qpsum", bufs=2, space="PSUM") as qpsum,
                ):
                    wq_sb = qp.tile([128, DC, D], F16)
                    nc.sync.dma_start(wq_sb, chunked(wq_d))
                    nc.sync.dma_start(wk_sb, chunked(wk_d))
                    for tj in range(1, 4):
                        nc.sync.dma_start(
                            xT[:, :, tj * 512:(tj + 1) * 512],
                            xt_ch[:, :, tj * 512:(tj + 1) * 512],
                        )
                    nc.sync.dma_start(wv_sb, chunked(wv_d))
                    nc.sync.dma_start(wp_sb, chunked(wp_d))
                    nc.sync.dma_start(bi_sb, bias_d)

                    # q^T[e, q] for this core's queries (x^T cols 0:512),
                    # written into the zero-padded layout.
                    for ej in range(DC):
                        qps = qpsum.tile([128, 512], F32, tag="qps")
                        for jd in range(DC):
                            nc.tensor.matmul(
                                qps,
                                lhsT=wq_sb[:, jd, ej * 128:(ej + 1) * 128],
                                rhs=xT[:, jd, 0:512],
                                start=(jd == 0),
                                stop=(jd == DC - 1),
                            )
                        eng = nc.scalar if ej % 2 == 0 else nc.vector
                        cp = (
                            nc.scalar.copy if ej % 2 == 0
                            else nc.vector.tensor_copy
                        )
                        for sel in range(2):
                            cp(
                                qpad[0:64, ej, sel, 0:QH],
                                qps[0:64, sel * QH:(sel + 1) * QH],
                            )
                            cp(
                                qpad[64:128, ej, sel, QH:2 * QH],
                                qps[64:128, sel * QH:(sel + 1) * QH],
                            )

                # ---------------- pass A: qh=0 + K/V production ----------
                with (
                    tc.tile_pool(name="accA", bufs=1, space="PSUM") as accp,
                    tc.tile_pool(name="scA", bufs=1, space="PSUM") as scp,
                    tc.tile_pool(name="kvps", bufs=2, space="PSUM") as kvp,
                    tc.tile_pool(name="Ep", bufs=2) as Ep,
                    tc.tile_pool(name="smx", bufs=2) as smx,
                ):
                    acc0 = accp.tile([128, D], F32)
                    acc1 = accp.tile([128, D], F32)
                    accs = [acc0, acc1]
                    # column-split accumulation groups share PSUM banks;
                    # start=True clears beyond its own columns on this HW,
                    # so pre-zero the banks and accumulate with start=False.
                    nc.vector.memset(acc0, 0.0)
                    nc.vector.memset(acc1, 0.0)
                    for kc in range(TC):
                        if kc % 4 == 0:
                            tj = kc // 4
                            for ej in range(DC):
                                kps = kvp.tile([128, 512], F32, tag="kv")
                                for jd in range(DC):
                                    nc.tensor.matmul(
                                        kps,
                                        lhsT=wk_sb[:, jd,
                                                   ej * 128:(ej + 1) * 128],
                                        rhs=xT[:, jd,
                                               tj * 512:(tj + 1) * 512],
                                        start=(jd == 0),
                                        stop=(jd == DC - 1),
                                    )
                                nc.scalar.copy(
                                    kT[:, ej, tj * 512:(tj + 1) * 512], kps
                                )
                        for eh in range(2):
                            vps = kvp.tile([128, 512], F32, tag="kv")
                            for jd in range(DC):
                                nc.tensor.matmul(
                                    vps,
                                    lhsT=xT[:, jd, kc * 128:(kc + 1) * 128],
                                    rhs=wv_sb[:, jd,
                                              eh * 512:(eh + 1) * 512],
                                    start=(jd == 0),
                                    stop=(jd == DC - 1),
                                )
                            if eh == 0:
                                nc.vector.tensor_copy(
                                    v_sb[:, kc, eh * 512:(eh + 1) * 512], vps
                                )
                            else:
                                nc.scalar.copy(
                                    v_sb[:, kc, eh * 512:(eh + 1) * 512], vps
                                )
                        _attend(nc, tc, scp, Ep, smx, kT, v_sb, qpad, accs,
                                kc, qh=0, pool_mult=False)
                    nc.scalar.copy(att[:, 0, :], acc0)
                    nc.scalar.copy(att[:, 1, :], acc1)

            # attn^T for qh0 via xbar DMA transpose (DMA engine only)
            for qc in range(2):
                nc.sync.dma_start_transpose(
                    aT[:, :, qc * 128:(qc + 1) * 128], att[:, qc, :]
                )

            # ---------------- pass B: qh=1 + qh0 projection ----------
            with (
                tc.tile_pool(name="accB", bufs=1, space="PSUM") as accpB,
                tc.tile_pool(name="scB", bufs=1, space="PSUM") as scpB,
                tc.tile_pool(name="prjps", bufs=2, space="PSUM") as prjp,
                tc.tile_pool(name="EpB", bufs=2) as EpB,
                tc.tile_pool(name="smxB", bufs=2) as smxB,
                tc.tile_pool(name="outp", bufs=2) as outp,
            ):
                def emit_proj(qs):
                    for eh in range(2):
                        pm = prjp.tile([128, 512], F32, tag="pm")
                        for jd in range(DC):
                            nc.tensor.matmul(
                                pm,
                                lhsT=aT[:, jd, qs * 128:(qs + 1) * 128],
                                rhs=wp_sb[:, jd, eh * 512:(eh + 1) * 512],
                                start=(jd == 0),
                                stop=(jd == DC - 1),
                            )
                        ot = outp.tile([128, 512], F32, tag="ot")
                        nc.vector.tensor_tensor(
                            ot, pm, bi_sb[:, eh * 512:(eh + 1) * 512], ADD
                        )
                        nc.sync.dma_start(
                            out_ch[:, qs, eh * 512:(eh + 1) * 512], ot
                        )

                accB0 = accpB.tile([128, D], F32)
                accB1 = accpB.tile([128, D], F32)
                accsB = [accB0, accB1]
                nc.vector.memset(accB0, 0.0)
                nc.vector.memset(accB1, 0.0)
                for kc in range(TC):
                    _attend(nc, tc, scpB, EpB, smxB, kT, v_sb, qpad, accsB,
                            kc, qh=1, pool_mult=True)
                    if kc == 2:
                        emit_proj(0)
                    elif kc == 6:
                        emit_proj(1)
                nc.scalar.copy(att[:, 0, :], accB0)
                nc.scalar.copy(att[:, 1, :], accB1)
                for qc in range(2):
                    nc.sync.dma_start_transpose(
                        aT[:, :, 256 + qc * 128: 256 + (qc + 1) * 128],
                        att[:, qc, :],
                    )
                emit_proj(2)
                emit_proj(3)

    nc.compile()
    return nc


def _attend(nc, tc, scp, Ep, smx, kT, v_sb, qpad, accs, kc, qh, pool_mult):
    """Scores + head-axis softmax + PV accumulation for one key chunk."""
    Et = Ep.tile([128, H, QH], F16, tag="E")
    for g in range(4):
        sc = scp.tile([128, 1024], F32, tag="sc")
        for i in range(2):
            pr = 2 * g + i
            nc.tensor.matmul(
                sc[:, i * 512:(i + 1) * 512],
                lhsT=kT[:, pr, kc * 128:(kc + 1) * 128],
                rhs=qpad[:, pr, qh, :],
                start=True,
                stop=True,
            )
        # fused PSUM evacuation + scale + exp
        nc.scalar.activation(Et[:, 4 * g:4 * g + 4, :], sc, EXP, scale=SCALE)
    # S = sum over heads (log tree), R = 1/S, P = E * R broadcast
    tmp = smx.tile([128, H // 2, QH], F16, tag="tmp")
    nc.vector.tensor_tensor(tmp, Et[:, 0:8], Et[:, 8:16], ADD)
    nc.vector.tensor_tensor(tmp[:, 0:4], tmp[:, 0:4], tmp[:, 4:8], ADD)
    nc.vector.tensor_tensor(tmp[:, 0:2], tmp[:, 0:2], tmp[:, 2:4], ADD)
    nc.vector.tensor_tensor(tmp[:, 0:1], tmp[:, 0:1], tmp[:, 1:2], ADD)
    r = smx.tile([128, 1, QH], F16, tag="r")
    with nc.allow_low_precision(
        reason="softmax denominator reciprocal in fp16"
    ):
        nc.vector.reciprocal(r, tmp[:, 0:1])
    if pool_mult:
        nc.vector.tensor_tensor(
            Et[:, 0:8], Et[:, 0:8], r.to_broadcast([128, 8, QH]), MULT
        )
        nc.gpsimd.tensor_tensor(
            Et[:, 8:16], Et[:, 8:16], r.to_broadcast([128, 8, QH]), MULT
        )
    else:
        nc.vector.tensor_tensor(
            Et, Et, r.to_broadcast([128, H, QH]), MULT
        )
    # PV: attn[q, d] orientation, PSUM accumulation across all key chunks
    for h in range(H):
        for qc in range(2):
            nc.tensor.matmul(
                accs[qc][:, h * HD:(h + 1) * HD],
                lhsT=Et[:, h, qc * 128:(qc + 1) * 128],
                rhs=v_sb[:, kc, h * HD:(h + 1) * HD],
                start=False,
                stop=(kc == TC - 1),
                skip_group_check=True,
            )


def get_nc():
    global _CACHED_NC
    if _CACHED_NC is None:
        _CACHED_NC = _build_nc()
    return _CACHED_NC


def kernel(x, w_qkv, w_proj, b_proj, _trace=False, _tmpdir=None):
    x = np.asarray(x, dtype=np.float32)
    w_qkv = np.asarray(w_qkv, dtype=np.float32)
    w_proj = np.asarray(w_proj, dtype=np.float32)
    b_proj = np.asarray(b_proj, dtype=np.float32)

    # Host-side layout prep: transpose + fp16 casts + per-core rotation.
    xT = [np.ascontiguousarray(x[b].T).astype(np.float16) for b in range(B)]
    wq = np.ascontiguousarray(w_qkv[:, 0:D]).astype(np.float16)
    wk = np.ascontiguousarray(w_qkv[:, D:2 * D]).astype(np.float16)
    wv = np.ascontiguousarray(w_qkv[:, 2 * D:3 * D]).astype(np.float16)
    wp = w_proj.astype(np.float16)
    bias = np.ascontiguousarray(
        np.broadcast_to(b_proj, (128, D))
    ).astype(np.float32)

    in_maps = []
    for c in range(NCORES):
        b = c // (NCORES // B)
        qofs = (c % (NCORES // B)) * QS
        xt_rot = np.ascontiguousarray(np.roll(xT[b], -qofs, axis=1))
        in_maps.append(
            {
                "xt": xt_rot,
                "wq": wq,
                "wk": wk,
                "wv": wv,
                "wp": wp,
                "bias": bias,
            }
        )

    nc = get_nc()
    res = bass_utils.run_bass_kernel_spmd(
        nc,
        in_maps,
        core_ids=list(range(NCORES)),
        trace=_trace,
        tmpdir=_tmpdir,
    )

    out = np.empty((B, T, D), dtype=np.float32)
    for c in range(NCORES):
        b = c // (NCORES // B)
        qofs = (c % (NCORES // B)) * QS
        out[b, qofs:qofs + QS] = res.results[c]["out"]
    if _trace:
        kernel._last_results = res
    return out


# revision 11
# speedup vs baseline: 1.3132x; 1.3091x over previous
"""Trainium2 Bass kernel for nn_Attention_46995532153449.

Module: qkv = x @ w_qkv; per-head scores = q k^T * hd^-0.5; softmax over the
HEAD axis (axis=1); attn = probs @ v; out = attn @ w_proj + b_proj.

Shapes: B=2, T=2048, D=1024, H=16, HD=64.

Sharding: data-parallel over (batch, query-block): core c handles batch c//4
and queries [(c%4)*512, (c%4+1)*512). The head-axis softmax is local (each
core holds all 16 heads for its query slice). K/V for the whole batch are
recomputed per core (collectives are priced far above their compute saving
by the cost model, so no cross-core exchange).

Structure (all chosen against the TimelineSim cost model):
  - host feeds x^T fp16 with columns ROTATED so the core's own 512 queries
    are columns 0:512 (one SPMD program, per-core data). Key order is a
    rotation, which attention is invariant to.
  - attention runs as two passes over the 16 key chunks (qh = 256-query
    halves) to fit PSUM. Pass A also produces K/V, software-pipelined as
    per-chunk lookahead filler (2 k^T tiles + 1 v tile per chunk) emitted
    BETWEEN a chunk's scores and its PV so the PE never stalls on the
    softmax chain; PV lags one chunk.
  - PV uses the attn[q, d] orientation: lhsT = P^T tile [keys, q] (M=128),
    rhs = v [keys, 64] (N=64) -> full 128-partition output at half the PE
    cycles of the attn^T[d, q] orientation. Accumulation runs over all 16
    key chunks directly in PSUM (pre-zeroed banks + start=False: column-
    split accumulation groups must not use start=True, which clears the
    whole partition row of a bank on this HW). No spill-adds.
  - attn[q, d] -> attn^T[d, q] via dma_start_transpose (xbar): DMA cycles
    only, no PE/ACT/DVE time, no PSUM.
  - softmax: exp on ScalarE (fused scale + PSUM evacuation, [128,1024]
    pieces, double-buffered so ACT pipelines with the scores matmuls);
    head-sum tree + reciprocal on VectorE; P = E*R split VectorE/GpSimd.
  - the output projection runs at the tail (PSUM is fully booked during
    pass B); its latency chain is short.
"""

import numpy as np

import concourse.bacc as bacc
import concourse.mybir as mybir
import concourse.tile as tile
from concourse import bass_utils

B, T, D, H = 2, 2048, 1024, 16
HD = D // H          # 64
SCALE = HD ** -0.5   # 0.125
NCORES = 8
QS = B * T // NCORES  # 512 queries per core
QH = QS // 2          # 256-query halves (PSUM budget)
DC = D // 128         # 8 d/e chunks of 128
TC = T // 128         # 16 key chunks of 128

F16 = mybir.dt.float16
F32 = mybir.dt.float32
ADD = mybir.AluOpType.add
MULT = mybir.AluOpType.mult
EXP = mybir.ActivationFunctionType.Exp

_CACHED_NC = None


def _build_nc():
    nc = bacc.Bacc(
        "TRN2", target_bir_lowering=False, debug=False, enable_asserts=False
    )

    xt_d = nc.dram_tensor("xt", [D, T], F16, kind="ExternalInput").ap()
    wq_d = nc.dram_tensor("wq", [D, D], F16, kind="ExternalInput").ap()
    wk_d = nc.dram_tensor("wk", [D, D], F16, kind="ExternalInput").ap()
    wv_d = nc.dram_tensor("wv", [D, D], F16, kind="ExternalInput").ap()
    wp_d = nc.dram_tensor("wp", [D, D], F16, kind="ExternalInput").ap()
    bias_d = nc.dram_tensor("bias", [128, D], F32, kind="ExternalInput").ap()
    out_d = nc.dram_tensor("out", [QS, D], F32, kind="ExternalOutput").ap()

    def chunked(ap):  # [(c p), f] -> [p, c, f]
        return ap.rearrange("(c p) f -> p c f", p=128)

    xt_ch = chunked(xt_d)
    wq_ch = chunked(wq_d)
    out_ch = chunked(out_d)

    with tile.TileContext(nc) as tc:
        with tc.tile_pool(name="persist", bufs=1) as pp:
            kT = pp.tile([128, DC, T], F16)      # k^T: [e, t], e-chunk major
            v_sb = pp.tile([128, TC, D], F16)    # v: [t, e], t-chunk major
            # zero-padded q^T: for head pair pr and query half qh, columns
            # [0:QH] hold head 2pr's q^T at partitions 0:64 (zeros below),
            # columns [QH:2QH] hold head 2pr+1's at partitions 64:128, so
            # every scores matmul is a full-128-partition K=128 matmul.
            qpad = pp.tile([128, DC, 2, 2 * QH], F16)
            att = pp.tile([128, 2, D], F16)      # attn [q, d], per-qh reuse
            aT = pp.tile([128, DC, QS], F16)     # attn^T [d, q]
            wp_sb = pp.tile([128, DC, D], F16)
            bi_sb = pp.tile([128, D], F32)

            nc.gpsimd.memset(qpad, 0.0)

            with tc.tile_pool(name="pA", bufs=1) as pA:
                xT = pA.tile([128, DC, T], F16)
                wk_sb = pA.tile([128, DC, D], F16)
                wv_sb = pA.tile([128, DC, D], F16)

                with (
                    tc.tile_pool(name="qpool", bufs=1) as qp,
                    tc.tile_pool(name="qpsum", bufs=2, space="PSUM") as qpsum,
                ):
                    wq_sb = qp.tile([128, DC, D], F16)
                    # DMA ring order: first half of wq -> own x^T piece ->
                    # rest of wq -> wk -> remaining x^T pieces -> wv -> wp
                    # -> bias. Q gates on the first three; K production on
                    # wk + successive x^T pieces; wp/bias only matter at
                    # the tail.
                    nc.sync.dma_start(wq_sb[:, :, 0:512], wq_ch[:, :, 0:512])
                    nc.sync.dma_start(xT[:, :, 0:512], xt_ch[:, :, 0:512])
                    nc.sync.dma_start(
                        wq_sb[:, :, 512:1024], wq_ch[:, :, 512:1024]
                    )
                    nc.sync.dma_start(wk_sb, chunked(wk_d))
                    for tj in range(1, 4):
                        nc.sync.dma_start(
                            xT[:, :, tj * 512:(tj + 1) * 512],
                            xt_ch[:, :, tj * 512:(tj + 1) * 512],
                        )
                    nc.sync.dma_start(wv_sb, chunked(wv_d))
                    nc.sync.dma_start(wp_sb, chunked(wp_d))
                    nc.sync.dma_start(bi_sb, bias_d)

                    # q^T[e, q] for this core's queries (x^T cols 0:512),
                    # written into the zero-padded layout.
                    for ej in range(DC):
                        qps = qpsum.tile([128, 512], F32, tag="qps")
                        for jd in range(DC):
                            nc.tensor.matmul(
                                qps,
                                lhsT=wq_sb[:, jd, ej * 128:(ej + 1) * 128],
                                rhs=xT[:, jd, 0:512],
                                start=(jd == 0),
                                stop=(jd == DC - 1),
                            )
                        cp = (
                            nc.scalar.copy if ej % 2 == 0
                            else nc.vector.tensor_copy
                        )
                        for sel in range(2):
                            cp(
                                qpad[0:64, ej, sel, 0:QH],
                                qps[0:64, sel * QH:(sel + 1) * QH],
                            )
                            cp(
                                qpad[64:128, ej, sel, QH:2 * QH],
                                qps[64:128, sel * QH:(sel + 1) * QH],
                            )

                # ---------------- pass A: qh=0 + K/V production ----------
                with (
                    tc.tile_pool(name="accA", bufs=1, space="PSUM") as accp,
                    tc.tile_pool(name="scA", bufs=1, space="PSUM") as scp,
                    tc.tile_pool(name="kvps", bufs=2, space="PSUM") as kvp,
                    tc.tile_pool(name="Ep", bufs=2) as Ep,
                    tc.tile_pool(name="smx", bufs=2) as smx,
                ):
                    def emit_k(tj, ej):
                        kps = kvp.tile([128, 512], F32, tag="kv")
                        for jd in range(DC):
                            nc.tensor.matmul(
                                kps,
                                lhsT=wk_sb[:, jd, ej * 128:(ej + 1) * 128],
                                rhs=xT[:, jd, tj * 512:(tj + 1) * 512],
                                start=(jd == 0),
                                stop=(jd == DC - 1),
                            )
                        nc.scalar.copy(
                            kT[:, ej, tj * 512:(tj + 1) * 512], kps
                        )

                    def emit_v(kc):
                        for eh in range(2):
                            vps = kvp.tile([128, 512], F32, tag="kv")
                            for jd in range(DC):
                                nc.tensor.matmul(
                                    vps,
                                    lhsT=xT[:, jd, kc * 128:(kc + 1) * 128],
                                    rhs=wv_sb[:, jd,
                                              eh * 512:(eh + 1) * 512],
                                    start=(jd == 0),
                                    stop=(jd == DC - 1),
                                )
                            cp = (
                                nc.vector.tensor_copy if eh == 0
                                else nc.scalar.copy
                            )
                            cp(v_sb[:, kc, eh * 512:(eh + 1) * 512], vps)

                    acc0 = accp.tile([128, D], F32)
                    acc1 = accp.tile([128, D], F32)
                    accs = [acc0, acc1]
                    # column-split accumulation groups share PSUM banks;
                    # start=True clears beyond its own columns on this HW,
                    # so pre-zero the banks and accumulate with start=False.
                    nc.vector.memset(acc0, 0.0)
                    nc.vector.memset(acc1, 0.0)

                    # prologue: k^T superstep 0 (keys 0:512) + v chunk 0
                    for ej in range(DC):
                        emit_k(0, ej)
                    emit_v(0)

                    pend = []  # softmax+PV closures, lagged one chunk
                    for kc in range(TC):
                        # K/V lookahead fillers, emitted BETWEEN score
                        # groups: the scores PSUM tile is single-buffered
                        # (bank budget), so group g+1's matmuls wait on
                        # group g's exp — the filler keeps the PE busy
                        # through that and through the softmax chain.
                        fillers = []
                        if kc < 12:
                            tj = kc // 4 + 1
                            fillers.append(
                                lambda tj=tj, e=2 * (kc % 4): emit_k(tj, e)
                            )
                            fillers.append(
                                lambda tj=tj, e=2 * (kc % 4) + 1:
                                emit_k(tj, e)
                            )
                        if kc < TC - 1:
                            fillers.append(lambda kc=kc: emit_v(kc + 1))
                        Et = _scores(nc, scp, Ep, kT, qpad, kc, qh=0,
                                     fillers=fillers)
                        pend.append(
                            lambda kc=kc, Et=Et: _softmax_pv(
                                nc, smx, v_sb, accs, Et, kc, dve_heads=10
                            )
                        )
                        if len(pend) > 1:
                            pend.pop(0)()
                    pend.pop(0)()
                    nc.scalar.copy(att[:, 0, :], acc0)
                    nc.vector.tensor_copy(att[:, 1, :], acc1)

            # attn^T for qh0 via xbar DMA transpose (DMA engine only)
            for qc in range(2):
                nc.sync.dma_start_transpose(
                    aT[:, :, qc * 128:(qc + 1) * 128], att[:, qc, :]
                )

            # ---------------- pass B: qh=1 ----------
            with (
                tc.tile_pool(name="accB", bufs=1, space="PSUM") as accpB,
                tc.tile_pool(name="scB", bufs=2, space="PSUM") as scpB,
                tc.tile_pool(name="EpB", bufs=4) as EpB,
                tc.tile_pool(name="smxB", bufs=2) as smxB,
            ):
                accB0 = accpB.tile([128, D], F32)
                accB1 = accpB.tile([128, D], F32)
                accsB = [accB0, accB1]
                nc.vector.memset(accB0, 0.0)
                nc.vector.memset(accB1, 0.0)
                pend = []  # PV lags two chunks: covers the softmax latency
                for kc in range(TC):
                    Et = _scores(nc, scpB, EpB, kT, qpad, kc, qh=1)
                    pend.append(
                        lambda kc=kc, Et=Et: _softmax_pv(
                            nc, smxB, v_sb, accsB, Et, kc, dve_heads=10
                        )
                    )
                    if len(pend) > 2:
                        pend.pop(0)()
                while pend:
                    pend.pop(0)()
                nc.scalar.copy(att[:, 0, :], accB0)
                nc.vector.tensor_copy(att[:, 1, :], accB1)
            for qc in range(2):
                nc.sync.dma_start_transpose(
                    aT[:, :, 256 + qc * 128:256 + (qc + 1) * 128],
                    att[:, qc, :],
                )

            # ---------------- tail: output projection ----------
            with (
                tc.tile_pool(name="prjps", bufs=4, space="PSUM") as prjp,
                tc.tile_pool(name="outp", bufs=4) as outp,
            ):
                for qs in range(4):
                    for eh in range(2):
                        pm = prjp.tile([128, 512], F32, tag="pm")
                        for jd in range(DC):
                            nc.tensor.matmul(
                                pm,
                                lhsT=aT[:, jd, qs * 128:(qs + 1) * 128],
                                rhs=wp_sb[:, jd, eh * 512:(eh + 1) * 512],
                                start=(jd == 0),
                                stop=(jd == DC - 1),
                            )
                        ot = outp.tile([128, 512], F32, tag="ot")
                        nc.vector.tensor_tensor(
                            ot, pm, bi_sb[:, eh * 512:(eh + 1) * 512], ADD
                        )
                        nc.sync.dma_start(
                            out_ch[:, qs, eh * 512:(eh + 1) * 512], ot
                        )

    nc.compile()
    return nc


def _scores(nc, scp, Ep, kT, qpad, kc, qh, fillers=()):
    """QK^T scores + fused scale/exp evacuation for one key chunk.

    `fillers` are emitted between score groups to give the PE independent
    work while the single-buffered scores tile round-trips through exp.
    """
    fillers = list(fillers)
    Et = Ep.tile([128, H, QH], F16, tag="E")
    for g in range(4):
        sc = scp.tile([128, 1024], F32, tag="sc")
        for i in range(2):
            pr = 2 * g + i
            nc.tensor.matmul(
                sc[:, i * 512:(i + 1) * 512],
                lhsT=kT[:, pr, kc * 128:(kc + 1) * 128],
                rhs=qpad[:, pr, qh, :],
                start=True,
                stop=True,
            )
        nc.scalar.activation(Et[:, 4 * g:4 * g + 4, :], sc, EXP, scale=SCALE)
        if fillers:
            fillers.pop(0)()
    while fillers:
        fillers.pop(0)()
    return Et


def _softmax_pv(nc, smx, v_sb, accs, Et, kc, dve_heads):
    """Head-axis softmax + PV accumulation for one key chunk."""
    # S = sum over heads (log tree), R = 1/S, P = E * R broadcast
    tmp = smx.tile([128, H // 2, QH], F16, tag="tmp")
    nc.vector.tensor_tensor(tmp, Et[:, 0:8], Et[:, 8:16], ADD)
    nc.vector.tensor_tensor(tmp[:, 0:4], tmp[:, 0:4], tmp[:, 4:8], ADD)
    nc.vector.tensor_tensor(tmp[:, 0:2], tmp[:, 0:2], tmp[:, 2:4], ADD)
    nc.vector.tensor_tensor(tmp[:, 0:1], tmp[:, 0:1], tmp[:, 1:2], ADD)
    r = smx.tile([128, 1, QH], F16, tag="r")
    with nc.allow_low_precision(
        reason="softmax denominator reciprocal in fp16"
    ):
        nc.vector.reciprocal(r, tmp[:, 0:1])
    a = dve_heads
    nc.vector.tensor_tensor(
        Et[:, 0:a], Et[:, 0:a], r.to_broadcast([128, a, QH]), MULT
    )
    nc.gpsimd.tensor_tensor(
        Et[:, a:H], Et[:, a:H], r.to_broadcast([128, H - a, QH]), MULT
    )
    # PV: attn[q, d] orientation, PSUM accumulation across all key chunks
    for h in range(H):
        for qc in range(2):
            nc.tensor.matmul(
                accs[qc][:, h * HD:(h + 1) * HD],
                lhsT=Et[:, h, qc * 128:(qc + 1) * 128],
                rhs=v_sb[:, kc, h * HD:(h + 1) * HD],
                start=False,
                stop=(kc == TC - 1),
                skip_group_check=True,
            )


def get_nc():
    global _CACHED_NC
    if _CACHED_NC is None:
        _CACHED_NC = _build_nc()
    return _CACHED_NC


def kernel(x, w_qkv, w_proj, b_proj, _trace=False, _tmpdir=None):
    x = np.asarray(x, dtype=np.float32)
    w_qkv = np.asarray(w_qkv, dtype=np.float32)
    w_proj = np.asarray(w_proj, dtype=np.float32)
    b_proj = np.asarray(b_proj, dtype=np.float32)

    # Host-side layout prep: transpose + fp16 casts + per-core rotation.
    xT = [np.ascontiguousarray(x[b].T).astype(np.float16) for b in range(B)]
    wq = np.ascontiguousarray(w_qkv[:, 0:D]).astype(np.float16)
    wk = np.ascontiguousarray(w_qkv[:, D:2 * D]).astype(np.float16)
    wv = np.ascontiguousarray(w_qkv[:, 2 * D:3 * D]).astype(np.float16)
    wp = w_proj.astype(np.float16)
    bias = np.ascontiguousarray(
        np.broadcast_to(b_proj, (128, D))
    ).astype(np.float32)

    in_maps = []
    for c in range(NCORES):
        b = c // (NCORES // B)
        qofs = (c % (NCORES // B)) * QS
        xt_rot = np.ascontiguousarray(np.roll(xT[b], -qofs, axis=1))
        in_maps.append(
            {
                "xt": xt_rot,
                "wq": wq,
                "wk": wk,
                "wv": wv,
                "wp": wp,
                "bias": bias,
            }
        )

    nc = get_nc()
    res = bass_utils.run_bass_kernel_spmd(
        nc,
        in_maps,
        core_ids=list(range(NCORES)),
        trace=_trace,
        tmpdir=_tmpdir,
    )

    out = np.empty((B, T, D), dtype=np.float32)
    for c in range(NCORES):
        b = c // (NCORES // B)
        qofs = (c % (NCORES // B)) * QS
        out[b, qofs:qofs + QS] = res.results[c]["out"]
    if _trace:
        kernel._last_results = res
    return out
